# revision 16
# baseline (speedup 1.0000x reference)
"""ACE symmetrizer kernel for one TRN2 chip (8 NeuronCores).

Computation (per (CG, A) pair): einsum('rm,fmn->rfn', CG, A) followed by
reshape(2L+1, -1, n).  Five A tensors x two CG sets (R=8 and R=24).

Strategy:
  - Data-parallel over the atom axis n (last dim): core c gets n-slice
    [c*128:(c+1)*128] of every A tensor; outputs are sharded the same way
    and concatenated on the host.  No collectives.
  - Host packs the ragged entries into two dense phases so every DMA and
    PSUM tile spans (close to) all 128 SBUF partitions:
      phase A: [A2_l11|A2_l22|A2_l33](f 0:512)  + A3_l211  -> K=83+45=128
      phase B: [A2_l11|A2_l22|A2_l33](f 512:1024) + A3_l111 -> K=83+27=110
    CG matrices (8+24 rows per entry) are stacked into one block-diagonal
    stationary operand per phase: [K, 128] (4 entries x 32 output rows).
  - Per f-block: SWDGE DMA [K, FB*128] to SBUF (gpsimd descriptors fan out
    across all 16 SDMA engines; HWDGE binds a DMA to one engine), matmuls
    of N=512 into PSUM [128, 512], PSUM -> SBUF copies alternating between
    the vector and scalar engines (DMA cannot read PSUM on TRN2), one DMA
    back to DRAM.
  - Inputs are cast to bf16 on the host (PE bf16 is 4x fp32 rate and it
    halves input DMA traffic); accumulation is fp32 in PSUM.
"""

import sys

for _p in ("/opt/trn_rl_repo",):
    if _p not in sys.path:
        sys.path.append(_p)

import numpy as np

import concourse.bass as bass  # noqa: F401  (registers engine namespaces)
import concourse.mybir as mybir
import concourse.tile as tile
from concourse import bacc
from concourse.bass_utils import run_bass_kernel_spmd

N_CORES = 8
N_FULL = 1024
NSH = N_FULL // N_CORES  # 128 atoms per core
FB = 64                  # f columns per input DMA block
FBO = 64                 # f columns per output DMA chunk
FP = 512                 # f columns per phase

# m-dims of the entries packed into each phase, in partition order.
# Phase A: l11, l22, l33 (f 0:512), l211.  Phase B: same G2 (f 512:1024), l111.
KA, KB = 128, 110
R = 128                  # output rows per phase: 4 entries x (8 + 24)

DT_IN = mybir.dt.bfloat16  # DRAM input + matmul dtype
DT_OUT = mybir.dt.bfloat16  # DRAM output dtype

_NP_DT = {mybir.dt.float32: np.float32}
try:
    import ml_dtypes
    _NP_DT[mybir.dt.bfloat16] = ml_dtypes.bfloat16
except ImportError:
    pass


def _build_nc(dt_in, dt_out):
    nc = bacc.Bacc("TRN2", target_bir_lowering=False, debug=False)
    ga = nc.dram_tensor("ga", [KA, FP, NSH], dt_in, kind="ExternalInput")
    gb = nc.dram_tensor("gb", [KB, FP, NSH], dt_in, kind="ExternalInput")
    wa = nc.dram_tensor("wa", [KA, R], dt_in, kind="ExternalInput")
    wb = nc.dram_tensor("wb", [KB, R], dt_in, kind="ExternalInput")
    oa = nc.dram_tensor("oa", [R, FP, NSH], dt_out, kind="ExternalOutput")
    ob = nc.dram_tensor("ob", [R, FP, NSH], dt_out, kind="ExternalOutput")

    with tile.TileContext(nc) as tc:
        with (
            tc.tile_pool(name="w", bufs=1) as wpool,
            tc.tile_pool(name="rhs", bufs=6) as rpool,
            tc.tile_pool(name="ost", bufs=6) as opool,
            tc.tile_pool(name="ps", bufs=8, space="PSUM") as ppool,
        ):
            wat = wpool.tile([KA, R], dt_in, tag="wa")
            wbt = wpool.tile([KB, R], dt_in, tag="wb")
            nc.sync.dma_start(wat[:], wa[:])
            nc.sync.dma_start(wbt[:], wb[:])
            blocks = [
                (g, wt, k_tot, o, fb)
                for g, wt, k_tot, o in ((ga, wat, KA, oa), (gb, wbt, KB, ob))
                for fb in range(FP // FB)
            ]
            IN_AHEAD = 6  # = rhs bufs; in-DMAs issued ahead of out-DMAs so
            #               Q7 head-of-line waits don't starve the SDMA engines

            def issue_in(i):
                g, wt, k_tot, o, fb = blocks[i]
                rt = rpool.tile([k_tot, FB * NSH], dt_in, tag="rhs")
                nc.gpsimd.dma_start(rt[:], g[:, fb * FB:(fb + 1) * FB, :])
                return rt

            rts = {i: issue_in(i) for i in range(min(IN_AHEAD, len(blocks)))}
            cnt = 0
            for i, (g, wt, k_tot, o, fb) in enumerate(blocks):
                rt = rts.pop(i)
                if i + IN_AHEAD < len(blocks):
                    rts[i + IN_AHEAD] = issue_in(i + IN_AHEAD)
                for h in range(FB // FBO):
                    ot = opool.tile([R, FBO * NSH], dt_out, tag="ost")
                    for j in range(FBO * NSH // 512):
                        col = h * FBO * NSH + j * 512
                        pt = ppool.tile([R, 512], mybir.dt.float32, tag="ps")
                        nc.tensor.matmul(
                            pt[:], wt[:], rt[:, col:col + 512],
                            start=True, stop=True,
                        )
                        if cnt % 2 == 0:
                            nc.vector.tensor_copy(ot[:, j * 512:(j + 1) * 512], pt[:])
                        else:
                            nc.scalar.copy(ot[:, j * 512:(j + 1) * 512], pt[:])
                        cnt += 1
                    f0 = fb * FB + h * FBO
                    nc.gpsimd.dma_start(o[:, f0:f0 + FBO, :], ot[:])
    nc.compile()
    return nc


_NC_CACHE = {}


def _get_nc():
    key = (DT_IN, DT_OUT)
    if key not in _NC_CACHE:
        _NC_CACHE[key] = _build_nc(*key)
    return _NC_CACHE[key]


# (entry, m, F) tables; phases pack [G2-entries (f-half), extra A3 entry].
_G2 = [("A2_l11", 9), ("A2_l22", 25), ("A2_l33", 49)]


def _host_prep(inputs):
    """Build per-core input maps from the full-size input dict."""
    np_in = _NP_DT[DT_IN]

    g2 = [np.asarray(inputs[n]).transpose(1, 0, 2) for n, _ in _G2]  # [m, 1024, n]
    l111 = np.asarray(inputs["A3_l111"]).transpose(1, 0, 2)          # [27, 512, n]
    l211 = np.asarray(inputs["A3_l211"]).transpose(1, 0, 2)          # [45, 512, n]

    ga_full = np.concatenate([t[:, :FP] for t in g2] + [l211], axis=0)   # [128, 512, 1024]
    gb_full = np.concatenate([t[:, FP:] for t in g2] + [l111], axis=0)   # [110, 512, 1024]

    def stack_w(k_tot, entries):
        # entries: list of (cg_suffix_prefix, m).  4 entries x 32 output rows.
        w = np.zeros((k_tot, R), np.float32)
        m0 = 0
        for e, (pref, suff, m) in enumerate(entries):
            cg0 = np.asarray(inputs[f"CG0_{pref}_{suff}"])  # [8, m]
            cg1 = np.asarray(inputs[f"CG1_{pref}_{suff}"])  # [24, m]
            w[m0:m0 + m, e * 32:e * 32 + 8] = cg0.T
            w[m0:m0 + m, e * 32 + 8:e * 32 + 32] = cg1.T
            m0 += m
        assert m0 == k_tot
        return w.astype(np_in)

    wa = stack_w(KA, [("2", "l11", 9), ("2", "l22", 25), ("2", "l33", 49),
                      ("3", "l211", 45)])
    wb = stack_w(KB, [("2", "l11", 9), ("2", "l22", 25), ("2", "l33", 49),
                      ("3", "l111", 27)])

    in_maps = []
    for c in range(N_CORES):
        sl = slice(c * NSH, (c + 1) * NSH)
        in_maps.append({
            "ga": np.ascontiguousarray(ga_full[:, :, sl]).astype(np_in, copy=False),
            "gb": np.ascontiguousarray(gb_full[:, :, sl]).astype(np_in, copy=False),
            "wa": wa,
            "wb": wb,
        })
    return in_maps


def _host_gather(results):
    """Reassemble the 10 full outputs from per-core oa/ob."""
    oa = np.concatenate([np.asarray(r["oa"]) for r in results], axis=2)
    ob = np.concatenate([np.asarray(r["ob"]) for r in results], axis=2)
    oa = oa.astype(np.float32, copy=False)  # [128, 512, 1024]
    ob = ob.astype(np.float32, copy=False)

    b0, b1 = [], []
    # G2 entries: rows e*32..(e+1)*32, f<512 in oa, f>=512 in ob.
    for e in range(3):
        rows = np.concatenate([oa[e * 32:(e + 1) * 32], ob[e * 32:(e + 1) * 32]],
                              axis=1)  # [32, 1024, 1024]
        b0.append(rows[:8].reshape(1, 8 * 1024, N_FULL))
        b1.append(rows[8:].reshape(3, 24 // 3 * 1024, N_FULL))
    # A3 entries: l111 = ob rows 96:128, l211 = oa rows 96:128.
    rows111 = ob[96:128]  # [32, 512, 1024]
    rows211 = oa[96:128]
    b0.append(rows111[:8].reshape(1, 8 * 512, N_FULL))
    b0.append(rows211[:8].reshape(1, 8 * 512, N_FULL))
    b1.append(rows111[8:].reshape(3, 8 * 512, N_FULL))
    b1.append(rows211[8:].reshape(3, 8 * 512, N_FULL))
    return tuple(b0) + tuple(b1)


def _run(inputs, trace=False, trace_cores=None):
    nc = _get_nc()
    in_maps = _host_prep(inputs)
    res = run_bass_kernel_spmd(
        nc, in_maps, core_ids=list(range(N_CORES)),
        trace=trace, trace_cores=trace_cores,
    )
    return _host_gather(res.results), res


def kernel(**inputs):
    outs, _ = _run(inputs)
    return outs


# revision 22
# speedup vs baseline: 1.1475x; 1.1475x over previous
"""ACE symmetrizer kernel for one TRN2 chip (8 NeuronCores).

Computation (per (CG, A) pair): einsum('rm,fmn->rfn', CG, A) followed by
reshape(2L+1, -1, n).  Five A tensors x two CG sets (R=8 and R=24).

Strategy:
  - Data-parallel over the atom axis n (last dim): core c gets n-slice
    [c*128:(c+1)*128] of every A tensor; outputs are sharded the same way
    and concatenated on the host.  No collectives.
  - Host packs the ragged entries into two dense phases so every DMA and
    PSUM tile spans (close to) all 128 SBUF partitions:
      phase A: [A2_l11|A2_l22|A2_l33](f 0:512)  + A3_l211  -> K=83+45=128
      phase B: [A2_l11|A2_l22|A2_l33](f 512:1024) + A3_l111 -> K=83+27=110
    CG matrices (8+24 rows per entry) are stacked into one block-diagonal
    stationary operand per phase: [K, 128] (4 entries x 32 output rows).
  - Per f-block: SWDGE DMA [K, FB*128] to SBUF (gpsimd descriptors fan out
    across all 16 SDMA engines; HWDGE binds a DMA to one engine), matmuls
    of N=512 into PSUM [128, 512], PSUM -> SBUF copies alternating between
    the vector and scalar engines (DMA cannot read PSUM on TRN2), one DMA
    back to DRAM.
  - Inputs are cast to bf16 on the host (PE bf16 is 4x fp32 rate and it
    halves input DMA traffic); accumulation is fp32 in PSUM.
"""

import sys

for _p in ("/opt/trn_rl_repo",):
    if _p not in sys.path:
        sys.path.append(_p)

import numpy as np

import concourse.bass as bass  # noqa: F401  (registers engine namespaces)
import concourse.mybir as mybir
import concourse.tile as tile
from concourse import bacc
from concourse.bass_utils import run_bass_kernel_spmd

N_CORES = 8
N_FULL = 1024
NSH = N_FULL // N_CORES  # 128 atoms per core
FB = 32                  # f columns per input DMA block
FBO = 32                 # f columns per output DMA chunk
FP = 512                 # f columns per phase

# m-dims of the entries packed into each phase, in partition order.
# Phase A: l11, l22, l33 (f 0:512), l211.  Phase B: same G2 (f 512:1024), l111.
KA, KB = 128, 110
R = 128                  # output rows per phase: 4 entries x (8 + 24)

DT_IN = mybir.dt.bfloat16  # DRAM input + matmul dtype
DT_OUT = mybir.dt.bfloat16  # DRAM output dtype

_NP_DT = {mybir.dt.float32: np.float32}
try:
    import ml_dtypes
    _NP_DT[mybir.dt.bfloat16] = ml_dtypes.bfloat16
except ImportError:
    pass


def _build_nc(dt_in, dt_out):
    nc = bacc.Bacc("TRN2", target_bir_lowering=False, debug=False)
    ga = nc.dram_tensor("ga", [KA, FP, NSH], dt_in, kind="ExternalInput")
    gb = nc.dram_tensor("gb", [KB, FP, NSH], dt_in, kind="ExternalInput")
    wa = nc.dram_tensor("wa", [KA, R], dt_in, kind="ExternalInput")
    wb = nc.dram_tensor("wb", [KB, R], dt_in, kind="ExternalInput")
    oa = nc.dram_tensor("oa", [R, FP, NSH], dt_out, kind="ExternalOutput")
    ob = nc.dram_tensor("ob", [R, FP, NSH], dt_out, kind="ExternalOutput")

    with tile.TileContext(nc) as tc:
        with (
            tc.tile_pool(name="w", bufs=1) as wpool,
            tc.tile_pool(name="rhs", bufs=12) as rpool,
            tc.tile_pool(name="ost", bufs=10) as opool,
            tc.tile_pool(name="ps", bufs=8, space="PSUM") as ppool,
        ):
            wat = wpool.tile([KA, R], dt_in, tag="wa")
            wbt = wpool.tile([KB, R], dt_in, tag="wb")
            nc.sync.dma_start(wat[:], wa[:])
            nc.sync.dma_start(wbt[:], wb[:])
            blocks = [
                (g, wt, k_tot, o, fb)
                for g, wt, k_tot, o in ((ga, wat, KA, oa), (gb, wbt, KB, ob))
                for fb in range(FP // FB)
            ]
            IN_AHEAD = 12  # = rhs bufs; in-DMAs issued ahead of out-DMAs so
            #               Q7 head-of-line waits don't starve the SDMA engines

            def issue_in(i):
                g, wt, k_tot, o, fb = blocks[i]
                rt = rpool.tile([k_tot, FB * NSH], dt_in, tag="rhs")
                nc.gpsimd.dma_start(rt[:], g[:, fb * FB:(fb + 1) * FB, :])
                return rt

            rts = {i: issue_in(i) for i in range(min(IN_AHEAD, len(blocks)))}
            cnt = 0
            for i, (g, wt, k_tot, o, fb) in enumerate(blocks):
                rt = rts.pop(i)
                if i + IN_AHEAD < len(blocks):
                    rts[i + IN_AHEAD] = issue_in(i + IN_AHEAD)
                for h in range(FB // FBO):
                    ot = opool.tile([R, FBO * NSH], dt_out, tag="ost")
                    for j in range(FBO * NSH // 512):
                        col = h * FBO * NSH + j * 512
                        pt = ppool.tile([R, 512], mybir.dt.float32, tag="ps")
                        nc.tensor.matmul(
                            pt[:], wt[:], rt[:, col:col + 512],
                            start=True, stop=True,
                        )
                        if cnt % 2 == 0:
                            nc.vector.tensor_copy(ot[:, j * 512:(j + 1) * 512], pt[:])
                        else:
                            nc.scalar.copy(ot[:, j * 512:(j + 1) * 512], pt[:])
                        cnt += 1
                    f0 = fb * FB + h * FBO
                    nc.gpsimd.dma_start(o[:, f0:f0 + FBO, :], ot[:])
    nc.compile()
    return nc


_NC_CACHE = {}


def _get_nc():
    key = (DT_IN, DT_OUT)
    if key not in _NC_CACHE:
        _NC_CACHE[key] = _build_nc(*key)
    return _NC_CACHE[key]


# (entry, m, F) tables; phases pack [G2-entries (f-half), extra A3 entry].
_G2 = [("A2_l11", 9), ("A2_l22", 25), ("A2_l33", 49)]


def _host_prep(inputs):
    """Build per-core input maps from the full-size input dict."""
    np_in = _NP_DT[DT_IN]

    g2 = [np.asarray(inputs[n]).transpose(1, 0, 2) for n, _ in _G2]  # [m, 1024, n]
    l111 = np.asarray(inputs["A3_l111"]).transpose(1, 0, 2)          # [27, 512, n]
    l211 = np.asarray(inputs["A3_l211"]).transpose(1, 0, 2)          # [45, 512, n]

    ga_full = np.concatenate([t[:, :FP] for t in g2] + [l211], axis=0)   # [128, 512, 1024]
    gb_full = np.concatenate([t[:, FP:] for t in g2] + [l111], axis=0)   # [110, 512, 1024]

    def stack_w(k_tot, entries):
        # entries: list of (cg_suffix_prefix, m).  4 entries x 32 output rows.
        w = np.zeros((k_tot, R), np.float32)
        m0 = 0
        for e, (pref, suff, m) in enumerate(entries):
            cg0 = np.asarray(inputs[f"CG0_{pref}_{suff}"])  # [8, m]
            cg1 = np.asarray(inputs[f"CG1_{pref}_{suff}"])  # [24, m]
            w[m0:m0 + m, e * 32:e * 32 + 8] = cg0.T
            w[m0:m0 + m, e * 32 + 8:e * 32 + 32] = cg1.T
            m0 += m
        assert m0 == k_tot
        return w.astype(np_in)

    wa = stack_w(KA, [("2", "l11", 9), ("2", "l22", 25), ("2", "l33", 49),
                      ("3", "l211", 45)])
    wb = stack_w(KB, [("2", "l11", 9), ("2", "l22", 25), ("2", "l33", 49),
                      ("3", "l111", 27)])

    in_maps = []
    for c in range(N_CORES):
        sl = slice(c * NSH, (c + 1) * NSH)
        in_maps.append({
            "ga": np.ascontiguousarray(ga_full[:, :, sl]).astype(np_in, copy=False),
            "gb": np.ascontiguousarray(gb_full[:, :, sl]).astype(np_in, copy=False),
            "wa": wa,
            "wb": wb,
        })
    return in_maps


def _host_gather(results):
    """Reassemble the 10 full outputs from per-core oa/ob."""
    oa = np.concatenate([np.asarray(r["oa"]) for r in results], axis=2)
    ob = np.concatenate([np.asarray(r["ob"]) for r in results], axis=2)
    oa = oa.astype(np.float32, copy=False)  # [128, 512, 1024]
    ob = ob.astype(np.float32, copy=False)

    b0, b1 = [], []
    # G2 entries: rows e*32..(e+1)*32, f<512 in oa, f>=512 in ob.
    for e in range(3):
        rows = np.concatenate([oa[e * 32:(e + 1) * 32], ob[e * 32:(e + 1) * 32]],
                              axis=1)  # [32, 1024, 1024]
        b0.append(rows[:8].reshape(1, 8 * 1024, N_FULL))
        b1.append(rows[8:].reshape(3, 24 // 3 * 1024, N_FULL))
    # A3 entries: l111 = ob rows 96:128, l211 = oa rows 96:128.
    rows111 = ob[96:128]  # [32, 512, 1024]
    rows211 = oa[96:128]
    b0.append(rows111[:8].reshape(1, 8 * 512, N_FULL))
    b0.append(rows211[:8].reshape(1, 8 * 512, N_FULL))
    b1.append(rows111[8:].reshape(3, 8 * 512, N_FULL))
    b1.append(rows211[8:].reshape(3, 8 * 512, N_FULL))
    return tuple(b0) + tuple(b1)


def _run(inputs, trace=False, trace_cores=None):
    nc = _get_nc()
    in_maps = _host_prep(inputs)
    last_err = None
    for _attempt in range(3):  # rare transient NRT_EXEC_UNIT_UNRECOVERABLE
        try:
            res = run_bass_kernel_spmd(
                nc, in_maps, core_ids=list(range(N_CORES)),
                trace=trace, trace_cores=trace_cores,
            )
            return _host_gather(res.results), res
        except Exception as e:  # noqa: BLE001
            last_err = e
    raise last_err


def kernel(**inputs):
    outs, _ = _run(inputs)
    return outs
